# revision 1
# baseline (speedup 1.0000x reference)
"""Trainium2 Bass kernel for nn_Conv2dKan (KAN-style 3x3 conv, 64->128 ch).

Math: out[b,o,l] = sum_k silu(u)*w_b + sum_{n,k} H_n(u)*(c*w_s), with u =
unfold(x) (3x3, pad 1). Linear in the basis functions, so the Hermite basis
H_0..H_7 is re-expressed in the monomial basis {silu(u), u, s=u^2, us, s^2,
us^2, s^3, us^3} with the basis change folded into the weights on the host.
H_0 == 1 and the even-polynomial constants contribute uniformly at every
output pixel (they also apply at zero-padding), so they fold into a per-o
bias. Device work per core (one batch item): a short ACT/DVE chain builds
8 feature planes in a zero-padded 50x50 layout, then an implicit GEMM:
9 shifted-window taps x 4 K-chunks of 128, PSUM-accumulated (fp32r).

Loop order is chunk-outer over all 5 output row-tiles (5 concurrent PSUM
banks) so the PE only ever waits for the first plane chunk and then runs
back-to-back, staying HAM-warm.

Sharding: batch 8 -> one image per NeuronCore, fully data parallel.
"""

import sys

if "/opt/trn_rl_repo" not in sys.path:
    sys.path.insert(0, "/opt/trn_rl_repo")

import numpy as np

import concourse.bacc as bacc
import concourse.bass as bass
import concourse.tile as tile
from concourse import mybir
from concourse.bass_utils import run_bass_kernel_spmd

# Problem constants (hardcoded per harness contract).
B = 8
C_IN = 64
C_OUT = 128
K = 3
N_BASIS = 8
H = W = 48
HP = WP = H + 2  # padded image
L = H * W
NTAPS = K * K
NCHUNK = 4  # four 128-row contraction chunks (8 planes x 64 ch)
# l-tiles: rows of the output image per PSUM tile (N = R*48 <= 512 fp32)
ROW_TILES = (10, 10, 10, 10, 8)

_CACHE = {}


def _build_program():
    nc = bacc.Bacc("TRN2", target_bir_lowering=False, debug=False, num_devices=1)
    f32 = mybir.dt.float32
    f32r = mybir.dt.float32r
    ACT = mybir.ActivationFunctionType

    x_d = nc.dram_tensor("x", [C_IN, HP * WP], f32, kind="ExternalInput").ap()
    xr_d = nc.dram_tensor("xr", [C_IN, HP * WP], f32r, kind="ExternalInput").ap()
    w_d = nc.dram_tensor("w", [128, NCHUNK * NTAPS * 128], f32r, kind="ExternalInput").ap()
    b_d = nc.dram_tensor("bias", [C_OUT, 1], f32, kind="ExternalInput").ap()
    o_d = nc.dram_tensor("out", [C_OUT, L], f32, kind="ExternalOutput").ap()

    PADN = HP * WP  # 2500 floats per partition per plane

    with tile.TileContext(nc) as tc:
        with (
            tc.tile_pool(name="big", bufs=1) as wpool,
            tc.tile_pool(name="outs", bufs=3) as opool,
            tc.tile_pool(name="psum", bufs=1, space="PSUM") as ppool,
        ):
            # ---- tiles ----
            w_sb = wpool.tile([128, NCHUNK * NTAPS * 128], f32r)
            bias_sb = wpool.tile([C_OUT, 1], f32)
            x_lo = wpool.tile([64, PADN], f32, tag="x_lo")  # x, partitions 0-63
            g = [wpool.tile([128, PADN], f32r, name=f"g{j}", tag=f"g{j}") for j in range(NCHUNK)]
            s_t = wpool.tile([128, PADN], f32, tag="s_t")   # [s | s]
            q_t = wpool.tile([128, PADN], f32, tag="q_t")   # [s2 | s2]

            xl_im = x_lo.rearrange("c (h w) -> c h w", h=HP)
            g_im = [t.rearrange("c (h w) -> c h w", h=HP) for t in g]
            g0f = g[0].bitcast(f32)  # u-plane readable as f32

            # ---- input DMAs first (per-ring issue order = priority) ----
            # x/xr arrive pre-padded from the host (contiguous transfers, no
            # on-chip border memsets; monomial pads stay exactly 0). Each
            # transfer is split across the 3 rings (sync/scalar/gpsimd).
            engines = (nc.sync, nc.scalar, nc.gpsimd)
            CS = (0, 834, 1667, PADN)  # column splits
            CW = NTAPS * 128
            WS = CW // 3

            def dma_x(b):
                engines[b].dma_start(
                    out=x_lo[:, CS[b] : CS[b + 1]], in_=x_d[:, CS[b] : CS[b + 1]]
                )

            def dma_xr(b):
                engines[b].dma_start(
                    out=g[0][64:128, CS[b] : CS[b + 1]],
                    in_=xr_d[:, CS[b] : CS[b + 1]],
                )

            def dma_w(j, b):
                c0 = j * CW + b * WS
                engines[b].dma_start(
                    out=w_sb[:, c0 : c0 + WS], in_=w_d[:, c0 : c0 + WS]
                )

            # scalar issues only its x/xr/wj0/wj1 slices, then computes;
            # its wj2/wj3 slices are issued between ACT compute ops below.
            # sync ring: the first conv matmul (row-tile 0) reads only g0
            # cols 0-599, so ship that xr prefix first and let wj0 slice 0
            # jump ahead of the xr remainder.
            for b in (0, 1, 2):
                dma_x(b)
            nc.sync.dma_start(out=g[0][64:128, 0:600], in_=xr_d[:, 0:600])
            dma_xr(1)
            dma_xr(2)
            dma_w(0, 0)
            nc.sync.dma_start(out=g[0][64:128, 600 : CS[1]], in_=xr_d[:, 600 : CS[1]])
            dma_w(0, 1)
            dma_w(0, 2)
            for j in range(1, NCHUNK):
                for b in (0, 2) if j >= 2 else (0, 1, 2):
                    dma_w(j, b)

            # ---- feature planes ----
            # ScalarE: silu over the full padded plane (silu(0)=0 to ~1e-8,
            # far below tolerance), then the squares; both sliced per DMA
            # column-slice so they start as soon as each slice lands
            for b in range(3):
                nc.scalar.activation(
                    g[0][0:64, CS[b] : CS[b + 1]], x_lo[:, CS[b] : CS[b + 1]], ACT.Silu
                )
            for b in range(3):
                nc.scalar.activation(
                    s_t[0:64, CS[b] : CS[b + 1]], x_lo[:, CS[b] : CS[b + 1]], ACT.Square
                )
            dma_w(2, 1)
            dma_w(3, 1)
            nc.scalar.dma_start(out=bias_sb[:], in_=b_d[:])
            # DVE: s upper from the u-plane, then products / copies
            nc.vector.tensor_mul(s_t[64:128], g0f[64:128], g0f[64:128])  # s (upper)
            nc.scalar.activation(q_t[:], s_t[:], ACT.Square)             # [s2|s2]
            nc.vector.tensor_mul(g[1][64:128], g0f[64:128], s_t[64:128])  # us
            nc.vector.tensor_copy(g[1][0:64], s_t[0:64])                  # s
            nc.vector.tensor_mul(g[2][64:128], g0f[64:128], q_t[64:128])  # us2
            nc.vector.tensor_copy(g[2][0:64], q_t[0:64])                  # s2
            nc.vector.tensor_mul(g[3][:], s_t[:], g[2].bitcast(f32)[:])   # [s3|us3]

            # ---- PE pre-warm: zero-matmuls into a scratch PSUM bank while
            # the input DMAs land, so HAM un-throttles (K=8/8, 2.4 GHz)
            # before the real stream starts ----
            warm = wpool.tile([128, 512], f32r, tag="warm")
            nc.vector.memset(warm.bitcast(f32)[:], 0.0)
            warm_ps = ppool.tile([128, 512], f32, tag="warm_ps")
            for _ in range(33):
                nc.tensor.matmul(
                    warm_ps[:], warm[:, 0:128], warm[:], start=True, stop=True
                )

            # ---- implicit GEMM: chunk-outer, all 5 row-tiles in flight ----
            psums = []
            h0s = []
            h0 = 0
            for R in ROW_TILES:
                psums.append(ppool.tile([128, R * W], f32, name=f"ps{h0}", tag=f"ps{len(h0s)}"))
                h0s.append(h0)
                h0 += R
            for j in range(NCHUNK):
                for it, R in enumerate(ROW_TILES):
                    h0 = h0s[it]
                    for dh in (-1, 0, 1):
                        for dw in (-1, 0, 1):
                            t9 = (dh + 1) * K + (dw + 1)
                            lhsT = w_sb[:, (j * NTAPS + t9) * 128 : (j * NTAPS + t9 + 1) * 128]
                            r0 = h0 + dh + 1
                            rhs = g_im[j][:, r0 : r0 + R, dw + 1 : dw + 1 + W]
                            nc.tensor.matmul(
                                psums[it][:],
                                lhsT,
                                rhs,
                                start=(j == 0 and t9 == 0),
                                stop=(j == NCHUNK - 1 and t9 == NTAPS - 1),
                            )
                    if j == NCHUNK - 1:
                        # evacuate with per-o bias add (ScalarE, PSUM->SBUF)
                        o_sb = opool.tile([C_OUT, R * W], f32, tag="osb")
                        if it < len(ROW_TILES) - 1:
                            nc.scalar.activation(
                                o_sb[:], psums[it][:], ACT.Identity, bias=bias_sb[:]
                            )
                            (nc.sync, nc.gpsimd, nc.sync, nc.gpsimd)[it].dma_start(
                                out=o_d[:, h0 * W : (h0 + R) * W], in_=o_sb[:]
                            )
                        else:
                            # last tile: halve evac+store so the final DMA
                            # starts sooner and the halves ride two rings
                            hn = R * W // 2
                            for hh, eng in ((0, nc.sync), (1, nc.gpsimd)):
                                nc.scalar.activation(
                                    o_sb[:, hh * hn : (hh + 1) * hn],
                                    psums[it][:, hh * hn : (hh + 1) * hn],
                                    ACT.Identity,
                                    bias=bias_sb[:],
                                )
                                eng.dma_start(
                                    out=o_d[
                                        :, h0 * W + hh * hn : h0 * W + (hh + 1) * hn
                                    ],
                                    in_=o_sb[:, hh * hn : (hh + 1) * hn],
                                )

    nc.compile()
    return nc


def _host_prep(w_b, w_s, c):
    """Fold Hermite->monomial basis change + w_s into the weights (fp64)."""
    wb = w_b[..., 0].astype(np.float64)          # (O, 576)
    cw = (c[..., 0] * w_s[None, ..., 0]).astype(np.float64)  # (N, O, 576)

    # monomial plane order: [silu, u, s, us, s2, us2, s3, us3]
    wm = np.zeros((8, C_OUT, C_IN * NTAPS), np.float64)
    wm[0] = wb
    wm[1] = 2 * cw[1] - 12 * cw[3] + 120 * cw[5] - 1680 * cw[7]
    wm[2] = 2 * cw[2] - 48 * cw[4] + 720 * cw[6]
    wm[3] = 8 * cw[3] - 160 * cw[5] + 3360 * cw[7]
    wm[4] = 16 * cw[4] - 480 * cw[6]
    wm[5] = 32 * cw[5] - 1344 * cw[7]
    wm[6] = 64 * cw[6]
    wm[7] = 128 * cw[7]
    bias = (cw[0] - 2 * cw[2] + 12 * cw[4] - 120 * cw[6]).sum(axis=1)  # (O,)

    # lhsT pack: [k_part=128, chunk=4, tap=9, o=128]
    # k_part = 64*half + c_in ; plane f = 2*chunk + half ; k = c_in*9 + tap
    wl = np.empty((128, NCHUNK, NTAPS, C_OUT), np.float32)
    cidx = np.arange(C_IN)
    for j in range(NCHUNK):
        for t in range(NTAPS):
            for half in range(2):
                f = 2 * j + half
                wl[64 * half : 64 * (half + 1), j, t, :] = (
                    wm[f][:, cidx * NTAPS + t].T.astype(np.float32)
                )
    # pre-round weights to the fp32r grid (sum of two bf16s)
    import ml_dtypes

    wlf = wl.reshape(128, NCHUNK * NTAPS * 128)
    hi = wlf.astype(ml_dtypes.bfloat16).astype(np.float32)
    lo = (wlf - hi).astype(ml_dtypes.bfloat16).astype(np.float32)
    wlf = hi + lo
    return wlf, bias.astype(np.float32).reshape(C_OUT, 1)


def _round_fp32r(a):
    import ml_dtypes

    hi = a.astype(ml_dtypes.bfloat16).astype(np.float32)
    lo = (a - hi).astype(ml_dtypes.bfloat16).astype(np.float32)
    return hi + lo


def _prep_in_maps(x, w_b, w_s, c):
    wl, bias = _host_prep(w_b, w_s, c)
    xi = np.asarray(x, np.float32)
    xp = np.zeros((B, C_IN, HP, WP), np.float32)
    xp[:, :, 1 : 1 + H, 1 : 1 + W] = xi
    xp = xp.reshape(B, C_IN, HP * WP)
    xr = _round_fp32r(xp)
    return [{"x": xp[i], "xr": xr[i], "w": wl, "bias": bias} for i in range(B)]


def kernel(x, w_b, w_s, c):
    if "nc" not in _CACHE:
        _CACHE["nc"] = _build_program()
    nc = _CACHE["nc"]

    in_maps = _prep_in_maps(x, w_b, w_s, c)
    res = run_bass_kernel_spmd(nc, in_maps, core_ids=list(range(B)))
    out = np.stack([res.results[i]["out"] for i in range(B)], axis=0)
    return out.reshape(B, C_OUT, H, W)

